# revision 24
# baseline (speedup 1.0000x reference)
"""Trainium2 Bass kernel for nn_Decision_Node (Linear+Hardtanh -> sp, 2-class
softmax Gini -> gini), data-parallel over 8 NeuronCores.

Math per core shard (B_s=128 of B=1024 batches, T=128, F=784, L=256, C=2):
    sp   = clip(x @ W.T + b, -1, 1)                      [N=16384, 256]
    gini = 2 - p0^2 - p1^2 = 1.5 - 0.5*tanh(sp*d/2)^2,  d = c[...,0]-c[...,1]

Device strategy (PE-clock-aware rewrite):
  - The PE p-state ramp only reaches 2.4 GHz under continuous execution, so
    every consumer is budgeted well below the 762 ns/row-tile PE stream rate
    and buffering is deep enough that the PE never stalls.
  - Host folds 127 into W and 127.5 into b, so PSUM holds 127*y + 127.5.
    One DVE tensor_scalar (max 0.5, min 254.5) -> u8 then implements
    hardtanh + round-to-nearest quantization of sp in a single pass over a
    full PSUM bank (two row-tiles at a time).
  - DVE scalar_tensor_tensor computes z = (sp_u8 - 127) * (d/254) = sp*d/2.
  - ACT: th = tanh(z); gini u8 = trunc((sqrt(255)*th)^2) = trunc(255*th^2).
  - Outputs are u8 (4.2 MB sp + 4.2 MB gini per core); host dequantizes
    sp = (u8-127)/127 and gini = 1.5 - (u8+0.5)/510.
  - Ramped block sizes (256 first) so the first matmul starts ~2 us in, and a
    small last group to shorten the pipeline drain tail.
  - x loads split across the sync (k 0..3 + remainder) and gpsimd (k 4,5)
    DMA queues; weights/d on scalar; stores on gpsimd.
"""

import os
import sys
import types
from concurrent.futures import ThreadPoolExecutor

import numpy as np

for _p in (
    "/opt/trn_rl_repo",
    "/root/.axon_site",
    "/root/.axon_site/_ro/trn_rl_repo",
    "/root/.axon_site/_ro/pypackages",
):
    if os.path.isdir(_p) and _p not in sys.path:
        sys.path.append(_p)

B, T, F, L = 1024, 128, 784, 256
NCORES = 8
BS = B // NCORES          # batches per core
NROWS = BS * T            # 16384 rows per core
KT = 7                    # contraction tiles (784 = 6*128 + 16, padded)
KP = 17                   # used partitions in the last (remainder+bias) k-tile
GRP = 8                   # row-tiles per stage group (full group)
NG = NROWS // (GRP * 128)  # output stage groups of 1024 rows (16)
GF = GRP * L              # free size of one full stage group (2048)
# Ramped blocks: small first blocks so the PE starts early, small last group
# to shorten the drain tail. Every block is a multiple of 256 rows and group
# boundaries never straddle a 1024-row output group.
BLOCKS = [256, 768, 1024] + [2048] * 6 + [1024, 768, 256]
assert sum(BLOCKS) == NROWS


def _build_module():
    """Build + compile the single-core Bass/Tile module (SPMD across cores)."""
    import concourse.tile as tile
    from concourse import bacc, mybir

    f32, f16, u8 = mybir.dt.float32, mybir.dt.float16, mybir.dt.uint8
    i8 = mybir.dt.int8
    Alu = mybir.AluOpType
    Act = mybir.ActivationFunctionType

    nc = bacc.Bacc(
        "TRN2",
        target_bir_lowering=False,
        debug=False,
        enable_asserts=False,
        num_devices=NCORES,
    )
    xt_d = nc.dram_tensor("xt", [KT, 128, NROWS], f16, kind="ExternalInput").ap()
    wt_d = nc.dram_tensor("wt", [KT, 128, L], f16, kind="ExternalInput").ap()
    d_d = nc.dram_tensor("d8", [T, GF], f16, kind="ExternalInput").ap()
    # Outputs staged as [group, partition, GRP*L] u8: 2 KiB linear per
    # partition per store; host de-interleaves + dequantizes.
    sp_d = nc.dram_tensor("sp", [NG, 128, GF], u8, kind="ExternalOutput").ap()
    gi_d = nc.dram_tensor("gini", [NG, 128, GF], u8, kind="ExternalOutput").ap()

    SQ255 = float(np.sqrt(255.0))

    with tile.TileContext(nc) as tc:
        with (
            tc.tile_pool(name="consts", bufs=1) as consts,
            tc.tile_pool(name="xt", bufs=5) as xt_pool,
            tc.tile_pool(name="psum", bufs=8, space="PSUM") as psum_pool,
            tc.tile_pool(name="stage", bufs=3) as stage_pool,
            tc.tile_pool(name="tmp", bufs=2) as tmp_pool,
        ):
            wt_sb = consts.tile([128, KT, L], f16)
            d8_sb = consts.tile([128, GF], f16)
            # d8 is first needed by the z-multiply (~6 us in); the software
            # DGE queue is idle until the first stores, so park it there and
            # keep the HW queues free for x.
            nc.gpsimd.dma_start(d8_sb[:], d_d[:])
            # Persistent last-k-tile buffers: rows 17..127 stay zero so the
            # matmul always contracts over 128 partitions (keeps FWL on);
            # only the 17 real rows are re-DMAed per block (double-buffered).
            xk6s = []
            for i in range(2):
                xk6 = consts.tile([128, 2048], f16, tag=f"xk6_{i}")
                nc.vector.memset(xk6[:], 0.0)
                xk6s.append(xk6)

            n0 = 0
            for bi, bnb in enumerate(BLOCKS):
                tpb = bnb // 128
                bgrp = min(GRP, tpb)   # row-tiles per stage group in block
                if bi < 2:
                    # Prologue blocks: one fused 3D DMA for all six full
                    # k-tiles -- a single ~0.6 us queue issue instead of six
                    # serial ones, so the first transfers start immediately.
                    xall = consts.tile([128, 6, bnb], f16, tag=f"xall{bi}")
                    # b0 on sync, b1 on scalar: the two HW queues deliver the
                    # first row blocks concurrently while the pipeline fills.
                    (nc.sync if bi == 0 else nc.scalar).dma_start(
                        xall[:],
                        xt_d[0:6, :, n0 : n0 + bnb].rearrange(
                            "k p n -> p k n"
                        ),
                    )
                    xts = [xall[:, k, :] for k in range(6)]
                else:
                    xts = []
                    for k in range(KT - 1):
                        xk = xt_pool.tile([128, 2048], f16, tag=f"x{k}")
                        # Sync carries the bulk of x (HW DGE). During the
                        # prologue (before ACT has tanh work) the scalar HW
                        # queue helps deliver x so the PE never starves while
                        # the pipeline fills; afterwards scalar only issues
                        # the small remainder tile. The gpsimd software DGE
                        # is reserved for stores.
                        xq = nc.scalar if (bi < 5 and k >= 4) else nc.sync
                        xq.dma_start(
                            xk[:, :bnb], xt_d[k, :, n0 : n0 + bnb]
                        )
                        xts.append(xk[:, :])
                xk6 = xk6s[bi % 2]
                nc.scalar.dma_start(
                    xk6[0:KP, :bnb], xt_d[KT - 1, 0:KP, n0 : n0 + bnb]
                )
                xts.append(xk6)
                if bi == 0:
                    # wt is only needed by the 7th matmul (~3 us in); issuing
                    # it after block 0's x keeps the first tiles' path clear.
                    nc.scalar.dma_start(
                        wt_sb[:], wt_d.rearrange("k p l -> p k l")
                    )
                for g in range(tpb // bgrp):
                    gf = bgrp * L
                    n0g = n0 + g * bgrp * 128
                    sp_st = stage_pool.tile([128, GRP, L], u8, tag="sp_st")
                    gi_st = stage_pool.tile([128, GRP, L], u8, tag="gi_st")
                    for hp in range(bgrp // 2):
                        ps = psum_pool.tile([128, 512], f32)
                        for h2 in range(2):
                            t = g * bgrp + hp * 2 + h2
                            for k in range(KT):
                                nc.tensor.matmul(
                                    ps[:, h2 * L : (h2 + 1) * L],
                                    xts[k][:, t * 128 : (t + 1) * 128],
                                    wt_sb[:, k, :],
                                    start=(k == 0),
                                    stop=(k == KT - 1),
                                )
                        # fused hardtanh + u8 quant: clip(127 y + 127.5,
                        # 0.5, 254.5) then truncate = round-to-nearest.
                        nc.vector.tensor_scalar(
                            sp_st[:, 2 * hp : 2 * hp + 2, :].rearrange(
                                "p a l -> p (a l)"
                            ),
                            ps[:],
                            0.5,
                            254.5,
                            Alu.max,
                            Alu.min,
                        )
                    sp_flat = sp_st[:, :bgrp, :].rearrange("p a l -> p (a l)")
                    gi_flat = gi_st[:, :bgrp, :].rearrange("p a l -> p (a l)")
                    z_big = tmp_pool.tile([128, GF], f16, tag="z")
                    th_big = tmp_pool.tile([128, GF], f16, tag="th")
                    # z = (sp_u8 - 127) * (d/254) = sp * d / 2
                    nc.vector.scalar_tensor_tensor(
                        z_big[:, :gf],
                        sp_flat,
                        127.0,
                        d8_sb[:, :gf],
                        Alu.subtract,
                        Alu.mult,
                    )
                    nc.scalar.activation(
                        th_big[:, :gf], z_big[:, :gf], Act.Tanh
                    )
                    # gini u8 = trunc((sqrt(255)*th)^2) = trunc(255*th^2)
                    nc.scalar.activation(
                        gi_flat, th_big[:, :gf], Act.Square, scale=SQ255
                    )
                    gidx = n0g // (GRP * 128)
                    c0 = (n0g % (GRP * 128)) // 128 * L
                    # Byte-sized outputs ride the software DGE, except the
                    # final blocks whose flush would otherwise gate teardown.
                    gq = nc.sync if bi >= len(BLOCKS) - 2 else nc.gpsimd
                    gq.dma_start(sp_d[gidx][:, c0 : c0 + gf], sp_flat)
                    gq.dma_start(gi_d[gidx][:, c0 : c0 + gf], gi_flat)
                n0 += bnb

    nc.compile()
    return nc


def _prep_core_x(x_flat_core):
    """[16384, 784] fp32 -> transposed fp16 [7, 128, 16384] (f on partitions).

    Row 16 of the last k-tile is the all-ones bias-fold row.
    """
    n = x_flat_core.shape[0]
    xsT16 = x_flat_core.T.astype(np.float16)  # [784, n], one strided pass
    xt = np.zeros((KT, 128, n), np.float16)
    xt[:6] = xsT16[:768].reshape(6, 128, n)
    xt[6, :16] = xsT16[768:784]
    xt[6, 16] = 1.0
    return xt


def _prep_wt(W, b):
    """W*127, bias row 127*b+127.5: PSUM = 127*y + 127.5 for u8 rounding."""
    wt = np.zeros((KT, 128, L), np.float16)
    WT = W.T * 127.0  # [784, 256]
    for k in range(6):
        wt[k] = WT[k * 128 : (k + 1) * 128]
    wt[6, :16] = WT[768:784]
    wt[6, 16] = 127.0 * b + 127.5
    return wt


_module_cache = {}


def _get_module():
    if "m" not in _module_cache:
        _module_cache["m"] = _build_module()
    return _module_cache["m"]


def _install_ntff_hook():
    """Register the axon NTFF profiling hook missing from this image's antenv."""
    try:
        import antenv.axon_hooks  # noqa: F401

        return
    except ImportError:
        pass
    try:
        from trn_agent_boot.trn_boot import _ntff_profile_via_ctypes

        hook = _ntff_profile_via_ctypes("/opt/axon/libaxon_pjrt.so")
    except Exception:
        hook = None
    mod = types.ModuleType("antenv.axon_hooks")
    mod.get_axon_ntff_profile_hook = lambda: hook
    mod.set_axon_ntff_profile_hook = lambda h: None
    sys.modules["antenv.axon_hooks"] = mod


def _run(x, W, b, contribution, trace=False, tmpdir=None):
    from concourse import bass_utils

    nc = _get_module()

    x_flat = np.ascontiguousarray(x, dtype=np.float32).reshape(NCORES, NROWS, F)
    wt = _prep_wt(np.asarray(W, np.float32), np.asarray(b, np.float32))
    c = np.asarray(contribution, np.float32)
    d = np.ascontiguousarray(c[:, :, 0] - c[:, :, 1], dtype=np.float32)
    d8 = np.ascontiguousarray(np.tile(d / 254.0, (1, GRP)).astype(np.float16))

    with ThreadPoolExecutor(NCORES) as ex:
        xts = list(ex.map(_prep_core_x, [x_flat[i] for i in range(NCORES)]))

    if trace:
        _install_ntff_hook()
    in_maps = [{"xt": xts[i], "wt": wt, "d8": d8} for i in range(NCORES)]
    res = bass_utils.run_bass_kernel_spmd(
        nc, in_maps, core_ids=list(range(NCORES)), trace=trace, tmpdir=tmpdir
    )

    def _unstage(raw):
        # [NG, 128, GRP*L] staged -> [nrows, L] row-major
        ng = raw.shape[0]
        return raw.reshape(ng, 128, GRP, L).swapaxes(1, 2).reshape(ng * 1024, L)

    sp_u = np.concatenate([_unstage(res.results[i]["sp"]) for i in range(NCORES)])
    gi_u = np.concatenate([_unstage(res.results[i]["gini"]) for i in range(NCORES)])
    sp = ((sp_u.astype(np.float32) - 127.0) / 127.0).reshape(B, T, L)
    gini = 1.5 - (gi_u.astype(np.float32) + 0.5) * (0.5 / 255.0)
    gini = gini.reshape(B, T, L)
    out = (sp, gini)
    return (out, res) if trace else (out, None)


def kernel(x, W, b, contribution):
    out, _ = _run(x, W, b, contribution, trace=False)
    return out
